# revision 6
# baseline (speedup 1.0000x reference)
"""2D DCT [8,32,256,256] on 8 TRN2 NeuronCores — raw Bass, even/odd folded.

Math: dct1d(x)[k] = (1/L) sum_m x[m] cos(pi k (m+0.5)/L).  Folding:
  dct[2j]   = sum_{m<128} (x[m]+x[255-m]) * Ae[m,j],  Ae[m,j]=cos(2pi j (m+.5)/256)/256
  dct[2j+1] = sum_{m<128} (x[m]-x[255-m]) * Ao[m,j],  Ao[m,j]=cos(pi (2j+1)(m+.5)/256)/256
halving the contraction (K=128 vs 256) of stage-1 matmuls.  Per slice:
  S1: V[w', i'] = sum_m {u|v}[m, w'] A{e|o}[m, i']   (4 matmuls K=M=N=128).
      Host ships u,v (stage-1 fold in numpy — same bytes as X) with the
      w axis pre-permuted to [0..127, 255..128], so V partition-chunk 1
      holds w-reversed rows and the stage-2 fold is chunk0 +/- chunk1.
  evictV: PSUM -> SBUF bf16 [128,512] copy (DVE/ACT alternating).
  S2: the stage-2 fold is folded INTO the matmuls (PSUM accumulation):
      out[i',j'] = Vc0^T @ [Ae|Ao] + Vc1^T @ [Ae|-Ao]  (8 matmuls N=128,
      4 accumulating pairs) — equivalent to (Vc0+Vc1)@Ae / (Vc0-Vc1)@Ao.
      (Elementwise two-PSUM-source tensor_tensor and all GpSimd tensor
      ops are rejected by neuronx-cc, so the fold must ride the PE.)
  evictOut: PSUM -> SBUF bf16, then SP/ACT DMA out.
Output rows/cols land even|odd-permuted; host unscrambles (free).

Measured on this part: back-to-back K=M=N=128 bf16 matmuls with distinct
stationaries pitch at 56 ns (LDWEIGHTS fully hidden; satisfied waits
free) -> PE ~= 32 * 12 * 56 = 21.5us vs 28us unfolded.  The binding
floor is DMA: 8.5 MB round trip at 360 GB/s/core (16 engines x 22.5
B/ns) ~= 23.5us; graduated input chunks issue up front on the SP HWDGE
ring, output chunks trail compute, keeping the 16 engines saturated.

Copy-engine colocation keeps PE waits to one per stage: evictV(s) and
evictOut(s-4) live on engine s%2 with deps 2s-3 < 2s-2, so S2(s)'s
single wait on evictOut(s-4)'s position also covers evictV(s).

Never let two agents touch one PSUM bank concurrently (PE-write +
reader, or two readers) — it hard-crashes the device.  Bank reuse is
gated through the PE waits below.
"""

import numpy as np

import concourse.bacc as bacc
import concourse.bass as bass
import concourse.mybir as mybir
from concourse.bass_utils import run_bass_kernel_spmd

N_CORES = 8
C = 32                    # slices per core
L = 256
BF16 = mybir.dt.bfloat16
F32 = mybir.dt.float32
NP_BF16 = mybir.dt.np(mybir.dt.bfloat16)

IN_CHUNKS = [2, 1, 1, 1, 2, 2, 3, 5, 8, 8]   # units (unit 0 = A tile)
OUT_CHUNKS = [6, 6, 6, 6, 4, 2]              # slices per SP out-DMA
TAIL_OUT = (30, 32)                          # ACT issues this inline
N_WARM = 20
PS_RV = 4                 # V PSUM ring (banks)
PS_RO = 4                 # out PSUM ring (banks)
VS_R = 6                  # evicted-V SBUF ring
LOOKAHEAD = PS_RV

PERM = np.concatenate([np.arange(0, 256, 2), np.arange(1, 256, 2)])
INV = np.argsort(PERM)
WSEQ = np.concatenate([np.arange(128), np.arange(255, 127, -1)])


def _dct_halves() -> tuple[np.ndarray, np.ndarray]:
    m = np.arange(128, dtype=np.float64)[:, None] + 0.5
    j = np.arange(128, dtype=np.float64)[None, :]
    ae = np.cos(2 * np.pi * j * m / L) / L
    ao = np.cos(np.pi * (2 * j + 1) * m / L) / L
    return ae.astype(np.float32), ao.astype(np.float32)


def _pe_schedule():
    order = []
    for s in range(C):
        order.append(("S1", s))
        if s >= LOOKAHEAD:
            order.append(("S2", s - LOOKAHEAD))
    for s in range(C - LOOKAHEAD, C):
        order.append(("S2", s))
    pe_count = {st: i + 1 for i, st in enumerate(order)}
    return order, pe_count


def _chunk_of_slice(s):
    u = s + 1
    c0 = 0
    for ci, n in enumerate(IN_CHUNKS):
        if u < c0 + n:
            return ci
        c0 += n
    raise AssertionError


def _copy_streams(pe_count):
    """Engine s%2 gets evictV(s) [dep S1(s)] and evictOut(s) [dep S2(s)].
    Returns per-parity dep-sorted event lists and pos[(kind, s)] -> 1-based
    index (== its engine sem value once done)."""
    streams = {0: [], 1: []}
    for s in range(C):
        streams[s % 2].append((pe_count[("S1", s)], "V", s))
        streams[s % 2].append((pe_count[("S2", s)], "O", s))
    pos = {}
    for p, evs in streams.items():
        evs.sort()
        for i, (dep, kind, s) in enumerate(evs):
            pos[(kind, s)] = i + 1
    return streams, pos


def _build() -> bass.Bass:
    nc = bacc.Bacc()
    x = nc.declare_dram_parameter("x", [128, C + 1, 2, L], BF16, isOutput=False)
    out = nc.declare_dram_parameter("out", [128, C, 2, L], BF16, isOutput=True)

    order, pe_count = _pe_schedule()
    cstreams, cpos = _copy_streams(pe_count)

    from contextlib import ExitStack

    ctx = ExitStack()
    with ctx:
        warm_sb = ctx.enter_context(nc.sbuf_tensor([128, 128], BF16))
        xs = ctx.enter_context(nc.sbuf_tensor([128, C + 1, 2, L], BF16))
        vs = ctx.enter_context(nc.sbuf_tensor([128, VS_R, 2, L], BF16))
        os_ = ctx.enter_context(nc.sbuf_tensor([128, C, 2, L], BF16))
        vp = ctx.enter_context(nc.psum_tensor([128, PS_RV, 2, L], F32))
        op = ctx.enter_context(nc.psum_tensor([128, PS_RO, 2, L], F32))

        in_sems = [
            ctx.enter_context(nc.semaphore(f"in_sem{i}"))
            for i in range(len(IN_CHUNKS))
        ]
        pe_sem = ctx.enter_context(nc.semaphore("pe_sem"))
        dve_sem = ctx.enter_context(nc.semaphore("dve_sem"))
        act_sem = ctx.enter_context(nc.semaphore("act_sem"))
        out_sem = ctx.enter_context(nc.semaphore("out_sem"))
        sem_of = {0: dve_sem, 1: act_sem}

        block = ctx.enter_context(nc.Block())

        @block.sync
        def _(eng):
            u0 = 0
            for ci, n in enumerate(IN_CHUNKS):
                eng.dma_start(
                    xs[:, u0 : u0 + n, :, :], x[:, u0 : u0 + n, :, :]
                ).then_inc(in_sems[ci], 16)
                u0 += n
            c0 = 0
            for n in OUT_CHUNKS:
                last = c0 + n - 1
                for p in (0, 1):
                    need = max(
                        (cpos[("O", s)] for s in range(c0, c0 + n) if s % 2 == p),
                        default=0,
                    )
                    if need:
                        eng.wait_ge(sem_of[p], need)
                eng.dma_start(
                    out[:, c0 : c0 + n, :, :], os_[:, c0 : c0 + n, :, :]
                ).then_inc(out_sem, 16)
                c0 += n
            eng.wait_ge(out_sem, 16 * (len(OUT_CHUNKS) + 1))

        @block.tensor
        def _(eng):
            for _ in range(N_WARM):
                nc.tensor.matmul(
                    vp[:, 0, 0, 0:128], warm_sb[:], warm_sb[:],
                    start=True, stop=True,
                )
            eng.wait_ge(in_sems[0], 16)   # A tile + slice 0
            seen_chunks = {0}
            for kind, s in order:
                if kind == "S1":
                    ci = _chunk_of_slice(s)
                    if ci not in seen_chunks:
                        seen_chunks.add(ci)
                        eng.wait_ge(in_sems[ci], 16)
                    if s >= PS_RV:
                        eng.wait_ge(sem_of[s % 2], cpos[("V", s - PS_RV)])
                    r = s % PS_RV
                    for eo in range(2):
                        for wc in range(2):
                            mm = nc.tensor.matmul(
                                vp[:, r, wc, eo * 128 : (eo + 1) * 128],
                                xs[:, 1 + s, eo, wc * 128 : (wc + 1) * 128],
                                xs[:, 0, 0, eo * 128 : (eo + 1) * 128],
                                start=True, stop=True,
                            )
                    mm.then_inc(pe_sem, 1)
                else:
                    if s >= PS_RO:
                        eng.wait_ge(sem_of[s % 2], cpos[("O", s - PS_RO)])
                    else:
                        eng.wait_ge(sem_of[s % 2], cpos[("V", s)])
                    r = s % PS_RO
                    for eo in range(2):
                        for ic in range(2):
                            o = op[:, r, ic, eo * 128 : (eo + 1) * 128]
                            nc.tensor.matmul(
                                o,
                                vs[:, s % VS_R, 0, ic * 128 : (ic + 1) * 128],
                                xs[:, 0, 0, eo * 128 : (eo + 1) * 128],
                                start=True, stop=False,
                            )
                            mm = nc.tensor.matmul(
                                o,
                                vs[:, s % VS_R, 1, ic * 128 : (ic + 1) * 128],
                                xs[:, 0, 1, eo * 128 : (eo + 1) * 128],
                                start=False, stop=True,
                            )
                    mm.then_inc(pe_sem, 1)

        def copy_stream(par):
            def body(eng):
                e = nc.vector if par == 0 else nc.scalar
                copy = e.tensor_copy if par == 0 else e.copy
                for dep, kind, s in cstreams[par]:
                    eng.wait_ge(pe_sem, dep)
                    if kind == "V":
                        copy(vs[:, s % VS_R, :, :], vp[:, s % PS_RV, :, :]).then_inc(
                            sem_of[par], 1
                        )
                    else:
                        copy(os_[:, s, :, :], op[:, s % PS_RO, :, :]).then_inc(
                            sem_of[par], 1
                        )
                if par == 1:
                    lo, hi = TAIL_OUT
                    for p in (0, 1):
                        need = max(
                            (cpos[("O", s)] for s in range(lo, hi) if s % 2 == p),
                            default=0,
                        )
                        if need:
                            eng.wait_ge(sem_of[p], need)
                    eng.dma_start(
                        out[:, lo:hi, :, :], os_[:, lo:hi, :, :]
                    ).then_inc(out_sem, 16)
            return body

        block.vector(copy_stream(0))
        block.scalar(copy_stream(1))

    nc.compile()
    return nc


_NC_CACHE: bass.Bass | None = None


def _get_nc() -> bass.Bass:
    global _NC_CACHE
    if _NC_CACHE is None:
        _NC_CACHE = _build()
    return _NC_CACHE


def _make_in_maps(ip: np.ndarray) -> list[dict[str, np.ndarray]]:
    ae, ao = _dct_halves()
    a_unit = np.zeros((128, 1, 2, L), np.float32)
    a_unit[:, 0, 0, 0:128] = ae
    a_unit[:, 0, 0, 128:256] = ao
    a_unit[:, 0, 1, 0:128] = ae
    a_unit[:, 0, 1, 128:256] = -ao
    a_unit = a_unit.astype(NP_BF16)

    xp = ip[:, :, :, WSEQ]                           # [8, C, 256, 256]
    u = xp[:, :, 0:128, :] + xp[:, :, :127:-1, :]    # [8, C, 128, 256]
    v = xp[:, :, 0:128, :] - xp[:, :, :127:-1, :]
    uv = np.stack([u, v], axis=2).astype(NP_BF16)    # [8, C, 2, 128, 256]

    in_maps = []
    for b in range(N_CORES):
        xb = uv[b].transpose(2, 0, 1, 3)             # [128, C, 2, 256]
        xb = np.concatenate([a_unit, xb], axis=1)    # [128, C+1, 2, 256]
        in_maps.append({"x": np.ascontiguousarray(xb)})
    return in_maps


def _unpack_out(results: list[dict[str, np.ndarray]]) -> np.ndarray:
    outs = []
    for b in range(N_CORES):
        ob = np.asarray(results[b]["out"]).astype(np.float32)   # [128, C, 2, L]
        ob = ob.transpose(1, 2, 0, 3).reshape(C, 256, 256)      # [c, t, col]
        outs.append(ob[:, INV, :][:, :, INV])
    return np.stack(outs, axis=0)


def run(ip: np.ndarray, trace: bool = False):
    ip = np.asarray(ip)
    assert ip.shape == (N_CORES, C, 256, 256), ip.shape
    res = run_bass_kernel_spmd(
        _get_nc(), _make_in_maps(ip), core_ids=list(range(N_CORES)), trace=trace
    )
    return _unpack_out(res.results), res


def kernel(ip: np.ndarray) -> np.ndarray:
    out, _ = run(ip)
    return out


# revision 7
# speedup vs baseline: 1.0324x; 1.0324x over previous
"""2D DCT [8,32,256,256] on 8 TRN2 NeuronCores — raw Bass, even/odd folded.

Math: dct1d(x)[k] = (1/L) sum_m x[m] cos(pi k (m+0.5)/L).  Folding:
  dct[2j]   = sum_{m<128} (x[m]+x[255-m]) * Ae[m,j],  Ae[m,j]=cos(2pi j (m+.5)/256)/256
  dct[2j+1] = sum_{m<128} (x[m]-x[255-m]) * Ao[m,j],  Ao[m,j]=cos(pi (2j+1)(m+.5)/256)/256
halving the contraction (K=128 vs 256) of stage-1 matmuls.  Per slice:
  S1: V[w', i'] = sum_m {u|v}[m, w'] A{e|o}[m, i']   (4 matmuls K=M=N=128).
      Host ships u,v (stage-1 fold in numpy — same bytes as X) with the
      w axis pre-permuted to [0..127, 255..128], so V partition-chunk 1
      holds w-reversed rows and the stage-2 fold is chunk0 +/- chunk1.
  evictV: PSUM -> SBUF bf16 [128,512] copy (DVE/ACT alternating).
  S2: the stage-2 fold is folded INTO the matmuls (PSUM accumulation):
      out[i',j'] = Vc0^T @ [Ae|Ao] + Vc1^T @ [Ae|-Ao]  (8 matmuls N=128,
      4 accumulating pairs) — equivalent to (Vc0+Vc1)@Ae / (Vc0-Vc1)@Ao.
      (Elementwise two-PSUM-source tensor_tensor and all GpSimd tensor
      ops are rejected by neuronx-cc, so the fold must ride the PE.)
  evictOut: PSUM -> SBUF bf16, then SP/ACT DMA out.
Output rows/cols land even|odd-permuted; host unscrambles (free).

Measured on this part: back-to-back K=M=N=128 bf16 matmuls with distinct
stationaries pitch at 56 ns (LDWEIGHTS fully hidden; satisfied waits
free) -> PE ~= 32 * 12 * 56 = 21.5us vs 28us unfolded.  The binding
floor is DMA: 8.5 MB round trip at 360 GB/s/core (16 engines x 22.5
B/ns) ~= 23.5us; graduated input chunks issue up front on the SP HWDGE
ring, output chunks trail compute, keeping the 16 engines saturated.

Copy-engine colocation keeps PE waits to one per stage: evictV(s) and
evictOut(s-4) live on engine s%2 with deps 2s-3 < 2s-2, so S2(s)'s
single wait on evictOut(s-4)'s position also covers evictV(s).

Never let two agents touch one PSUM bank concurrently (PE-write +
reader, or two readers) — it hard-crashes the device.  Bank reuse is
gated through the PE waits below.
"""

import numpy as np

import concourse.bacc as bacc
import concourse.bass as bass
import concourse.mybir as mybir
from concourse.bass_utils import run_bass_kernel_spmd

N_CORES = 8
C = 32                    # slices per core
L = 256
BF16 = mybir.dt.bfloat16
F32 = mybir.dt.float32
NP_BF16 = mybir.dt.np(mybir.dt.bfloat16)

IN_CHUNKS = [2, 1, 1, 1, 2, 2, 3, 5, 8, 8]   # units (unit 0 = A tile)
OUT_CHUNKS = [6, 6, 6, 6, 4, 2, 1]           # slices per SP out-DMA
TAIL_OUT = (31, 32)                          # ACT issues this inline
N_WARM = 10
PS_RV = 4                 # V PSUM ring (banks)
PS_RO = 4                 # out PSUM ring (banks)
VS_R = 6                  # evicted-V SBUF ring
LOOKAHEAD = PS_RV

PERM = np.concatenate([np.arange(0, 256, 2), np.arange(1, 256, 2)])
INV = np.argsort(PERM)
WSEQ = np.concatenate([np.arange(128), np.arange(255, 127, -1)])


def _dct_halves() -> tuple[np.ndarray, np.ndarray]:
    m = np.arange(128, dtype=np.float64)[:, None] + 0.5
    j = np.arange(128, dtype=np.float64)[None, :]
    ae = np.cos(2 * np.pi * j * m / L) / L
    ao = np.cos(np.pi * (2 * j + 1) * m / L) / L
    return ae.astype(np.float32), ao.astype(np.float32)


def _pe_schedule():
    order = []
    for s in range(C):
        order.append(("S1", s))
        if s >= LOOKAHEAD:
            order.append(("S2", s - LOOKAHEAD))
    for s in range(C - LOOKAHEAD, C):
        order.append(("S2", s))
    pe_count = {st: i + 1 for i, st in enumerate(order)}
    return order, pe_count


def _chunk_of_slice(s):
    u = s + 1
    c0 = 0
    for ci, n in enumerate(IN_CHUNKS):
        if u < c0 + n:
            return ci
        c0 += n
    raise AssertionError


def _copy_streams(pe_count):
    """Engine s%2 gets evictV(s) [dep S1(s)] and evictOut(s) [dep S2(s)].
    Returns per-parity dep-sorted event lists and pos[(kind, s)] -> 1-based
    index (== its engine sem value once done)."""
    streams = {0: [], 1: []}
    for s in range(C):
        streams[s % 2].append((pe_count[("S1", s)], "V", s))
        streams[s % 2].append((pe_count[("S2", s)], "O", s))
    pos = {}
    for p, evs in streams.items():
        evs.sort()
        for i, (dep, kind, s) in enumerate(evs):
            pos[(kind, s)] = i + 1
    return streams, pos


def _build() -> bass.Bass:
    nc = bacc.Bacc()
    x = nc.declare_dram_parameter("x", [128, C + 1, 2, L], BF16, isOutput=False)
    out = nc.declare_dram_parameter("out", [128, C, 2, L], BF16, isOutput=True)

    order, pe_count = _pe_schedule()
    cstreams, cpos = _copy_streams(pe_count)

    from contextlib import ExitStack

    ctx = ExitStack()
    with ctx:
        warm_sb = ctx.enter_context(nc.sbuf_tensor([128, 128], BF16))
        xs = ctx.enter_context(nc.sbuf_tensor([128, C + 1, 2, L], BF16))
        vs = ctx.enter_context(nc.sbuf_tensor([128, VS_R, 2, L], BF16))
        os_ = ctx.enter_context(nc.sbuf_tensor([128, C, 2, L], BF16))
        vp = ctx.enter_context(nc.psum_tensor([128, PS_RV, 2, L], F32))
        op = ctx.enter_context(nc.psum_tensor([128, PS_RO, 2, L], F32))

        in_sems = [
            ctx.enter_context(nc.semaphore(f"in_sem{i}"))
            for i in range(len(IN_CHUNKS))
        ]
        pe_sem = ctx.enter_context(nc.semaphore("pe_sem"))
        dve_sem = ctx.enter_context(nc.semaphore("dve_sem"))
        act_sem = ctx.enter_context(nc.semaphore("act_sem"))
        out_sem = ctx.enter_context(nc.semaphore("out_sem"))
        sem_of = {0: dve_sem, 1: act_sem}

        block = ctx.enter_context(nc.Block())

        @block.sync
        def _(eng):
            u0 = 0
            for ci, n in enumerate(IN_CHUNKS):
                eng.dma_start(
                    xs[:, u0 : u0 + n, :, :], x[:, u0 : u0 + n, :, :]
                ).then_inc(in_sems[ci], 16)
                u0 += n
            c0 = 0
            for n in OUT_CHUNKS:
                last = c0 + n - 1
                for p in (0, 1):
                    need = max(
                        (cpos[("O", s)] for s in range(c0, c0 + n) if s % 2 == p),
                        default=0,
                    )
                    if need:
                        eng.wait_ge(sem_of[p], need)
                eng.dma_start(
                    out[:, c0 : c0 + n, :, :], os_[:, c0 : c0 + n, :, :]
                ).then_inc(out_sem, 16)
                c0 += n
            eng.wait_ge(out_sem, 16 * (len(OUT_CHUNKS) + 1))

        @block.tensor
        def _(eng):
            for _ in range(N_WARM):
                nc.tensor.matmul(
                    vp[:, 0, 0, 0:128], warm_sb[:], warm_sb[:],
                    start=True, stop=True,
                )
            eng.wait_ge(in_sems[0], 16)   # A tile + slice 0
            seen_chunks = {0}
            for kind, s in order:
                if kind == "S1":
                    ci = _chunk_of_slice(s)
                    if ci not in seen_chunks:
                        seen_chunks.add(ci)
                        eng.wait_ge(in_sems[ci], 16)
                    if s >= PS_RV:
                        eng.wait_ge(sem_of[s % 2], cpos[("V", s - PS_RV)])
                    r = s % PS_RV
                    for eo in range(2):
                        for wc in range(2):
                            mm = nc.tensor.matmul(
                                vp[:, r, wc, eo * 128 : (eo + 1) * 128],
                                xs[:, 1 + s, eo, wc * 128 : (wc + 1) * 128],
                                xs[:, 0, 0, eo * 128 : (eo + 1) * 128],
                                start=True, stop=True,
                            )
                    mm.then_inc(pe_sem, 1)
                else:
                    if s >= PS_RO:
                        eng.wait_ge(sem_of[s % 2], cpos[("O", s - PS_RO)])
                    else:
                        eng.wait_ge(sem_of[s % 2], cpos[("V", s)])
                    r = s % PS_RO
                    for eo in range(2):
                        for ic in range(2):
                            o = op[:, r, ic, eo * 128 : (eo + 1) * 128]
                            nc.tensor.matmul(
                                o,
                                vs[:, s % VS_R, 0, ic * 128 : (ic + 1) * 128],
                                xs[:, 0, 0, eo * 128 : (eo + 1) * 128],
                                start=True, stop=False,
                            )
                            mm = nc.tensor.matmul(
                                o,
                                vs[:, s % VS_R, 1, ic * 128 : (ic + 1) * 128],
                                xs[:, 0, 1, eo * 128 : (eo + 1) * 128],
                                start=False, stop=True,
                            )
                    mm.then_inc(pe_sem, 1)

        def copy_stream(par):
            def body(eng):
                e = nc.vector if par == 0 else nc.scalar
                copy = e.tensor_copy if par == 0 else e.copy
                for dep, kind, s in cstreams[par]:
                    eng.wait_ge(pe_sem, dep)
                    if kind == "V":
                        copy(vs[:, s % VS_R, :, :], vp[:, s % PS_RV, :, :]).then_inc(
                            sem_of[par], 1
                        )
                    else:
                        copy(os_[:, s, :, :], op[:, s % PS_RO, :, :]).then_inc(
                            sem_of[par], 1
                        )
                if par == 1:
                    lo, hi = TAIL_OUT
                    for p in (0, 1):
                        need = max(
                            (cpos[("O", s)] for s in range(lo, hi) if s % 2 == p),
                            default=0,
                        )
                        if need:
                            eng.wait_ge(sem_of[p], need)
                    eng.dma_start(
                        out[:, lo:hi, :, :], os_[:, lo:hi, :, :]
                    ).then_inc(out_sem, 16)
            return body

        block.vector(copy_stream(0))
        block.scalar(copy_stream(1))

    nc.compile()
    return nc


_NC_CACHE: bass.Bass | None = None


def _get_nc() -> bass.Bass:
    global _NC_CACHE
    if _NC_CACHE is None:
        _NC_CACHE = _build()
    return _NC_CACHE


def _make_in_maps(ip: np.ndarray) -> list[dict[str, np.ndarray]]:
    ae, ao = _dct_halves()
    a_unit = np.zeros((128, 1, 2, L), np.float32)
    a_unit[:, 0, 0, 0:128] = ae
    a_unit[:, 0, 0, 128:256] = ao
    a_unit[:, 0, 1, 0:128] = ae
    a_unit[:, 0, 1, 128:256] = -ao
    a_unit = a_unit.astype(NP_BF16)

    xp = ip[:, :, :, WSEQ]                           # [8, C, 256, 256]
    u = xp[:, :, 0:128, :] + xp[:, :, :127:-1, :]    # [8, C, 128, 256]
    v = xp[:, :, 0:128, :] - xp[:, :, :127:-1, :]
    uv = np.stack([u, v], axis=2).astype(NP_BF16)    # [8, C, 2, 128, 256]

    in_maps = []
    for b in range(N_CORES):
        xb = uv[b].transpose(2, 0, 1, 3)             # [128, C, 2, 256]
        xb = np.concatenate([a_unit, xb], axis=1)    # [128, C+1, 2, 256]
        in_maps.append({"x": np.ascontiguousarray(xb)})
    return in_maps


def _unpack_out(results: list[dict[str, np.ndarray]]) -> np.ndarray:
    outs = []
    for b in range(N_CORES):
        ob = np.asarray(results[b]["out"]).astype(np.float32)   # [128, C, 2, L]
        ob = ob.transpose(1, 2, 0, 3).reshape(C, 256, 256)      # [c, t, col]
        outs.append(ob[:, INV, :][:, :, INV])
    return np.stack(outs, axis=0)


def run(ip: np.ndarray, trace: bool = False):
    ip = np.asarray(ip)
    assert ip.shape == (N_CORES, C, 256, 256), ip.shape
    res = run_bass_kernel_spmd(
        _get_nc(), _make_in_maps(ip), core_ids=list(range(N_CORES)), trace=trace
    )
    return _unpack_out(res.results), res


def kernel(ip: np.ndarray) -> np.ndarray:
    out, _ = run(ip)
    return out
